# revision 25
# baseline (speedup 1.0000x reference)
"""Trainium2 Bass kernel for nn_GruAgent (GRU + actor/critic MLP heads).

Strategy: the `done` flags reset the GRU state, cutting every env's
512-step timeline into independent segments (mean length ~20, max ~184
for the reference inputs).  On the host we bin-pack all segments into
8*C lanes of length N (~195), which shortens the sequential recurrence
from 512 chain steps to ~195 while widening each step's tiles.  The
per-core kernel runs the recurrence in a transposed layout
[features, lanes] with bf16 matmuls/gates, the input projection and the
actor/critic heads overlapped with the sequential chain.  The host
permutes x/done in, un-permutes the outputs; this is exact (resets make
segments independent), not an approximation.

Self-contained: hardcodes all shapes; only depends on the platform's
concourse (Bass) library.
"""

import math
import os
import sys

import numpy as np

for _p in ("/opt/trn_rl_repo", os.path.expanduser("~/.axon_site/_ro/trn_rl_repo")):
    if os.path.isdir(_p) and _p not in sys.path:
        sys.path.insert(0, _p)
        break

import ml_dtypes

import concourse.bass as bass
import concourse.mybir as mybir
import concourse.tile as tile
from concourse import bacc

T, B, OBS, H, A, L = 512, 512, 64, 64, 6, 64
N_CORES = 8
C = 170                     # lanes (columns) per core
GS = 3                      # chain steps per group (GS*C <= 512 psum fp32)
COLS = GS * C               # 510
H3 = 3 * H
NLANES = N_CORES * C        # 1360
AO = A + 1                  # 7 outputs (6 logits + 1 value)

F32 = mybir.dt.float32
BF16 = mybir.dt.bfloat16
AF = mybir.ActivationFunctionType
ALU = mybir.AluOpType
BF = ml_dtypes.bfloat16

WEIGHT_KEYS = [
    "w_ih", "w_hh", "b_ih", "b_hh",
    "aw1", "ab1", "aw2", "ab2", "aw3", "ab3",
    "cw1", "cb1", "cw2", "cb2", "cw3", "cb3",
]


# --------------------------------------------------------------------------
# device kernel
# --------------------------------------------------------------------------

def build(nc, ng):
    """Emit the per-core kernel for ng groups (N = ng*GS chain steps)."""
    from contextlib import ExitStack

    n_steps = ng * GS

    xT_d = nc.dram_tensor("xT", [ng, OBS, COLS], BF16, kind="ExternalInput")
    mb_d = nc.dram_tensor("mb", [ng, H, COLS], BF16, kind="ExternalInput")
    h0T_d = nc.dram_tensor("h0T", [H, C], BF16, kind="ExternalInput")
    wih_d = nc.dram_tensor("w_ihT", [OBS, H3], BF16, kind="ExternalInput")
    whh_d = nc.dram_tensor("w_hhT", [H, H3], BF16, kind="ExternalInput")
    l1h_d = nc.dram_tensor("l1h", [64, 128], BF16, kind="ExternalInput")
    l1x_d = nc.dram_tensor("l1x", [64, 128], BF16, kind="ExternalInput")
    l2_d = nc.dram_tensor("l2", [128, 128], BF16, kind="ExternalInput")
    l3_d = nc.dram_tensor("l3", [128, AO], BF16, kind="ExternalInput")
    brz_d = nc.dram_tensor("b_rz", [128, 1], F32, kind="ExternalInput")
    bhhn_d = nc.dram_tensor("b_hhn", [H, 1], F32, kind="ExternalInput")
    bihn_d = nc.dram_tensor("b_ihn", [H, 1], F32, kind="ExternalInput")
    b1_d = nc.dram_tensor("b1", [128, 1], F32, kind="ExternalInput")
    b2_d = nc.dram_tensor("b2", [128, 1], F32, kind="ExternalInput")
    b3_d = nc.dram_tensor("b3", [AO, 1], F32, kind="ExternalInput")
    b3r_d = nc.dram_tensor("b3r", [1, AO], BF16, kind="ExternalInput")
    out_d = nc.dram_tensor("out", [ng, AO, COLS], F32, kind="ExternalOutput")

    with tile.TileContext(nc) as tc, ExitStack() as ctx:
        wp = ctx.enter_context(tc.tile_pool(name="wp", bufs=1))
        catp = ctx.enter_context(tc.tile_pool(name="catp", bufs=3))
        mbp = ctx.enter_context(tc.tile_pool(name="mbp", bufs=3))

        gatep = ctx.enter_context(tc.tile_pool(name="gatep", bufs=2))
        smallp = ctx.enter_context(tc.tile_pool(name="smallp", bufs=2))
        tmlp = ctx.enter_context(tc.tile_pool(name="tmlp", bufs=3))
        o7p = ctx.enter_context(tc.tile_pool(name="o7p", bufs=2))

        przp = ctx.enter_context(tc.tile_pool(name="przp", bufs=2, space="PSUM"))
        pginp = ctx.enter_context(tc.tile_pool(name="pginp", bufs=2, space="PSUM"))
        pghnp = ctx.enter_context(tc.tile_pool(name="pghnp", bufs=1, space="PSUM"))
        phd = ctx.enter_context(tc.tile_pool(name="phd", bufs=3, space="PSUM"))

        # ---- weights / biases (host-prepared, straight DMA loads) ----
        def load(dram, shape, dt, tag):
            t = wp.tile(shape, dt, tag=tag)
            nc.sync.dma_start(t[:], dram[:])
            return t

        wihT = load(wih_d, [OBS, H3], BF16, "wihT")
        whhT = load(whh_d, [H, H3], BF16, "whhT")
        h0T = load(h0T_d, [H, C], BF16, "h0T")
        l1h = load(l1h_d, [64, 128], BF16, "l1h")
        l1x = load(l1x_d, [64, 128], BF16, "l1x")
        l2 = load(l2_d, [128, 128], BF16, "l2")
        l3 = load(l3_d, [128, AO], BF16, "l3")
        b_r = wp.tile([H, 1], F32, tag="b_r")
        nc.sync.dma_start(b_r[:], brz_d[0:64])
        b_z = wp.tile([H, 1], F32, tag="b_z")
        nc.sync.dma_start(b_z[:], brz_d[64:128])
        b_hhn = load(bhhn_d, [H, 1], F32, "b_hhn")
        b_ihn = load(bihn_d, [H, 1], F32, "b_ihn")
        b1 = load(b1_d, [128, 1], F32, "b1")
        b2 = load(b2_d, [128, 1], F32, "b2")
        b3r = load(b3r_d, [1, AO], BF16, "b3r")
        ones_row = wp.tile([1, COLS], BF16, tag="ones_row")
        nc.vector.memset(ones_row[:], 1.0)

        def bulk(g):
            """x + mask loads, input-projection preloads for group g."""
            xT = catp.tile([OBS, COLS], BF16, tag="xT")
            nc.sync.dma_start(xT[:], xT_d[g])
            hsg = catp.tile([H, COLS], BF16, tag="hsg")
            mbt = mbp.tile([H, COLS], BF16, tag="mb")
            nc.sync.dma_start(mbt[:], mb_d[g])
            prz = przp.tile([128, COLS], F32, tag="prz")
            nc.tensor.matmul(
                prz[:], wihT[:, 0:128], xT[:],
                start=True, stop=False, skip_group_check=True,
            )
            pgin = pginp.tile([H, COLS], F32, tag="pgin")
            nc.tensor.matmul(
                pgin[:], wihT[:, 128:H3], xT[:], start=True, stop=True
            )
            return dict(xT=xT, hsg=hsg, mb=mbt, prz=prz, pgin=pgin)

        state = {}

        def chain(g, refs, refs_next, pieces=None):
            hsg, mbt, prz, pgin = refs["hsg"], refs["mb"], refs["prz"], refs["pgin"]
            for s in range(GS):
                t = g * GS + s
                cs = bass.ts(s, C)
                mh = state["mh"]
                nc.tensor.matmul(
                    prz[:, cs], whhT[:, 0:128], mh[:],
                    start=False, stop=(s == GS - 1), skip_group_check=True,
                )
                pghn = pghnp.tile([H, C], F32, tag="pghn")
                nc.tensor.matmul(
                    pghn[:], whhT[:, 128:H3], mh[:], start=True, stop=True
                )
                r = gatep.tile([H, C], BF16, tag="r")
                nc.scalar.activation(r[:], prz[0:64, cs], AF.Sigmoid, bias=b_r[:])
                z = gatep.tile([H, C], BF16, tag="z")
                nc.scalar.activation(z[:], prz[64:128, cs], AF.Sigmoid, bias=b_z[:])
                if pieces is not None:
                    pieces[s][0]()          # ACT-side head work rides the p/q gap
                zm1 = smallp.tile([H, C], BF16, tag="zm1")
                nc.gpsimd.tensor_mul(zm1[:], z[:], mh[:])
                p = smallp.tile([H, C], BF16, tag="p")
                nc.vector.scalar_tensor_tensor(
                    p[:], pghn[:], b_hhn[:], r[:], ALU.add, ALU.mult
                )
                q = smallp.tile([H, C], BF16, tag="q")
                nc.vector.tensor_add(q[:], p[:], pgin[:, cs])
                n = smallp.tile([H, C], BF16, tag="n")
                nc.scalar.activation(n[:], q[:], AF.Tanh, bias=b_ihn[:])
                vb = smallp.tile([H, C], BF16, tag="vb")
                nc.vector.scalar_tensor_tensor(
                    vb[:], z[:], 1.0, n[:], ALU.subtract, ALU.mult
                )
                nc.vector.tensor_sub(hsg[:, cs], zm1[:], vb[:])
                if t < n_steps - 1:
                    mh2 = smallp.tile([H, C], BF16, tag="mh")
                    if s == GS - 1:
                        mbn = refs_next["mb"][:, 0:C]
                    else:
                        mbn = mbt[:, bass.ts(s + 1, C)]
                    nc.vector.tensor_mul(mh2[:], hsg[:, cs], mbn)
                    state["mh"] = mh2
                if pieces is not None:
                    pieces[s][1]()          # matmul/DMA-side head work after the step

        def head_pieces(g, refs):
            """Head MLP for group g as GS (act_part, mm_part) pairs."""
            hsg, xT = refs["hsg"], refs["xT"]
            st = {}

            def mm0():
                p1 = phd.tile([128, COLS], F32, tag="ph")
                nc.tensor.matmul(p1[:], l1h[:], hsg[:], start=True, stop=False,
                                 skip_group_check=True)
                nc.tensor.matmul(p1[:], l1x[:], xT[:], start=False, stop=True,
                                 skip_group_check=True)
                st["p1"] = p1

            def act1():
                t1 = tmlp.tile([128, COLS], BF16, tag="t1")
                nc.scalar.activation(t1[:], st["p1"][:], AF.Tanh, bias=b1[:])
                st["t1"] = t1

            def mm1():
                p2 = phd.tile([128, COLS], F32, tag="ph")
                nc.tensor.matmul(p2[:], l2[:], st["t1"][:], start=True, stop=True)
                st["p2"] = p2

            def act2():
                t2 = tmlp.tile([128, COLS], BF16, tag="t2")
                nc.scalar.activation(t2[:], st["p2"][:], AF.Tanh, bias=b2[:])
                st["t2"] = t2

            def mm2():
                p3 = phd.tile([128, COLS], F32, tag="ph")
                nc.tensor.matmul(p3[:AO, :], b3r[:], ones_row[:],
                                 start=True, stop=False, skip_group_check=True)
                nc.tensor.matmul(p3[:AO, :], l3[:], st["t2"][:],
                                 start=False, stop=True, skip_group_check=True)
                o7 = o7p.tile([AO, COLS], F32, tag="o7")
                half = COLS // 2
                nc.vector.tensor_copy(o7[:, 0:half], p3[:AO, 0:half])
                nc.vector.tensor_copy(o7[:, half:COLS], p3[:AO, half:COLS])
                nc.sync.dma_start(out_d[g], o7[:])

            def nop():
                pass

            return [(nop, mm0), (act1, mm1), (act2, mm2)]

        refs = bulk(0)
        mh0 = smallp.tile([H, C], BF16, tag="mh")
        nc.vector.tensor_mul(mh0[:], h0T[:], refs["mb"][:, 0:C])
        state["mh"] = mh0
        pieces = None
        for g in range(1, ng):
            refs_next = bulk(g)
            chain(g - 1, refs, refs_next, pieces)
            pieces = head_pieces(g - 1, refs)
            refs = refs_next
        chain(ng - 1, refs, None, pieces)
        for ap, mp in head_pieces(ng - 1, refs):
            ap()
            mp()

    return nc


_BUILT = {}


def get_built(ng):
    if ng not in _BUILT:
        nc = bacc.Bacc(None, target_bir_lowering=False)
        build(nc, ng)
        nc.compile()
        _BUILT[ng] = nc
    return _BUILT[ng]


# --------------------------------------------------------------------------
# host-side packing
# --------------------------------------------------------------------------

def _enumerate_segments(done2):
    """done2 [T,B] -> (seg_env, seg_t0, seg_len) with cuts at done==1.0."""
    starts = done2 == 1.0
    starts[0, :] = True
    nseg_per_env = starts.sum(axis=0)
    seg_env = np.repeat(np.arange(done2.shape[1]), nseg_per_env)
    env_idx, t_idx = np.nonzero(starts.T)
    seg_t0 = t_idx
    # length = next start - this start (within env)
    seg_len = np.empty(len(seg_t0), np.int64)
    pos = 0
    Tn = done2.shape[0]
    for b, k in enumerate(nseg_per_env):
        ts = seg_t0[pos:pos + k]
        seg_len[pos:pos + k - 1] = np.diff(ts)
        seg_len[pos + k - 1] = Tn - ts[-1]
        pos += k
    return seg_env, seg_t0, seg_len


def _pack(done2, h0_nonzero):
    """Bin-pack segments into NLANES lanes.  Returns (N, lane, off) per seg."""
    seg_env, seg_t0, seg_len = _enumerate_segments(done2)
    nseg = len(seg_len)
    total = int(seg_len.sum())
    cap = max(int(seg_len.max()), math.ceil(total / NLANES))
    cap = ((cap + GS - 1) // GS) * GS

    while True:
        rem = np.full(NLANES, cap, np.int64)
        lane = np.full(nseg, -1, np.int64)
        off = np.zeros(nseg, np.int64)
        ok = True
        if h0_nonzero:
            # t=0 segments that continue from h0 must sit at a lane start
            pin = np.nonzero((seg_t0 == 0) & (done2[0, seg_env] != 1.0))[0]
            if len(pin) > NLANES:
                raise RuntimeError("too many h0 segments")
            for j, i in enumerate(pin):
                if seg_len[i] > cap:
                    ok = False
                    break
                lane[i] = j
                off[i] = 0
                rem[j] = cap - seg_len[i]
        if ok:
            order = np.argsort(-seg_len, kind="stable")
            for i in order:
                if lane[i] >= 0:
                    continue
                Lg = seg_len[i]
                j = int(np.argmax(rem >= Lg))
                if rem[j] < Lg:
                    ok = False
                    break
                lane[i] = j
                off[i] = cap - rem[j]
                rem[j] -= Lg
        if ok:
            return cap, seg_env, seg_t0, seg_len, lane, off
        cap += GS


def _prepare(inputs):
    """Host-side pack + permute.  Returns (ng, in_maps, gather_idx)."""
    x = np.ascontiguousarray(np.asarray(inputs["x"], np.float32)).reshape(T, B, OBS)
    done2 = np.ascontiguousarray(
        np.asarray(inputs["done"], np.float32)
    ).reshape(T, B)
    h0 = np.ascontiguousarray(
        np.asarray(inputs["gru_state"], np.float32)
    ).reshape(B, H)
    h0_nonzero = bool(np.any(h0))

    N, seg_env, seg_t0, seg_len, seg_lane, seg_off = _pack(done2, h0_nonzero)
    ng = N // GS

    # flat (src slot) -> (dst slot) index arrays
    reps = seg_len
    src_env = np.repeat(seg_env, reps)
    within = np.concatenate([np.arange(l) for l in seg_len])
    src_t = np.repeat(seg_t0, reps) + within
    dst_lane = np.repeat(seg_lane, reps)
    dst_n = np.repeat(seg_off, reps) + within

    # packed done: copy source done values; padding slots = 1 (reset)
    donep = np.ones((N, NLANES), np.float32)
    donep[dst_n, dst_lane] = done2[src_t, src_env]
    if not h0_nonzero:
        # zero h0 == reset; break any dependence on prior lane garbage
        startmask = within == 0
        donep[dst_n[startmask], dst_lane[startmask]] = 1.0
    mbp_ = (1.0 - donep).astype(BF)                       # [N, NLANES]

    xp = np.zeros((N, NLANES, OBS), BF)
    xp[dst_n, dst_lane] = x[src_t, src_env].astype(BF)

    h0T_all = np.zeros((H, NLANES), BF)
    if h0_nonzero:
        first = within == 0
        fl = dst_lane[first]
        fe = src_env[first]
        fn = dst_n[first]
        sel = fn == 0
        h0T_all[:, fl[sel]] = h0[fe[sel]].T.astype(BF)

    # weights, host-transformed
    w_ih = np.asarray(inputs["w_ih"], np.float32)
    w_hh = np.asarray(inputs["w_hh"], np.float32)
    b_ih = np.asarray(inputs["b_ih"], np.float32)
    b_hh = np.asarray(inputs["b_hh"], np.float32)
    l1 = np.concatenate(
        [np.asarray(inputs["aw1"], np.float32).T,
         np.asarray(inputs["cw1"], np.float32).T], axis=1)       # [128,128]
    l2 = np.zeros((128, 128), np.float32)
    l2[0:64, 0:64] = np.asarray(inputs["aw2"], np.float32).T
    l2[64:128, 64:128] = np.asarray(inputs["cw2"], np.float32).T
    l3 = np.zeros((128, AO), np.float32)
    l3[0:64, 0:A] = np.asarray(inputs["aw3"], np.float32).T
    l3[64:128, A:AO] = np.asarray(inputs["cw3"], np.float32).T
    brz = (b_ih[0:128] + b_hh[0:128]).reshape(128, 1)
    bhhn = b_hh[128:H3].reshape(H, 1)
    bihn = b_ih[128:H3].reshape(H, 1)
    b1 = np.concatenate(
        [np.asarray(inputs["ab1"], np.float32),
         np.asarray(inputs["cb1"], np.float32)]).reshape(128, 1)
    b2 = np.concatenate(
        [np.asarray(inputs["ab2"], np.float32),
         np.asarray(inputs["cb2"], np.float32)]).reshape(128, 1)
    b3 = np.concatenate(
        [np.asarray(inputs["ab3"], np.float32),
         np.asarray(inputs["cb3"], np.float32)]).reshape(AO, 1)

    common = {
        "b3r": np.ascontiguousarray(b3.reshape(1, AO).astype(BF)),
        "w_ihT": np.ascontiguousarray(w_ih.T.astype(BF)),
        "w_hhT": np.ascontiguousarray(w_hh.T.astype(BF)),
        "l1h": np.ascontiguousarray(l1[0:64].astype(BF)),
        "l1x": np.ascontiguousarray(l1[64:128].astype(BF)),
        "l2": l2.astype(BF), "l3": l3.astype(BF),
        "b_rz": brz, "b_hhn": bhhn, "b_ihn": bihn,
        "b1": b1, "b2": b2, "b3": b3,
    }

    in_maps = []
    for c in range(N_CORES):
        sl = slice(c * C, (c + 1) * C)
        # [N, C, OBS] -> [ng, OBS, GS*C] with column order (s, lane)
        xc = xp[:, sl, :].reshape(ng, GS, C, OBS).transpose(0, 3, 1, 2)
        mc = mbp_[:, sl].reshape(ng, GS, C)
        mcb = np.broadcast_to(mc[:, None, :, :], (ng, H, GS, C))
        m = dict(common)
        m["xT"] = np.ascontiguousarray(xc.reshape(ng, OBS, COLS))
        m["mb"] = np.ascontiguousarray(mcb.reshape(ng, H, COLS))
        m["h0T"] = np.ascontiguousarray(h0T_all[:, sl])
        in_maps.append(m)

    gather = (src_t, src_env, dst_n, dst_lane, N, ng)
    return ng, in_maps, gather


def _assemble(per_core_outs, gather):
    src_t, src_env, dst_n, dst_lane, N, ng = gather
    # per-core out [ng, AO, COLS] -> [N, C, AO]
    packed = np.concatenate(
        [
            np.asarray(o, np.float32)
            .reshape(ng, AO, GS, C)
            .transpose(0, 2, 3, 1)
            .reshape(N, C, AO)
            for o in per_core_outs
        ],
        axis=1,
    )                                                     # [N, NLANES, AO]
    full = np.empty((T * B, AO), np.float32)
    full[src_t * B + src_env] = packed[dst_n, dst_lane]
    return full


def run_on_hw(inputs, trace=False, **kw):
    from concourse.bass_utils import run_bass_kernel_spmd

    ng, in_maps, gather = _prepare(inputs)
    nc = get_built(ng)
    res = run_bass_kernel_spmd(
        nc, in_maps, core_ids=list(range(N_CORES)), trace=trace, **kw
    )
    out = _assemble([r["out"] for r in res.results], gather)
    return out, res


def kernel(**inputs):
    out, _ = run_on_hw(inputs)
    return out


# revision 29
# speedup vs baseline: 1.1952x; 1.1952x over previous
"""Trainium2 Bass kernel for nn_GruAgent (GRU + actor/critic MLP heads).

Strategy: the `done` flags reset the GRU state, cutting every env's
512-step timeline into independent segments (mean length ~20, max ~184
for the reference inputs).  On the host we bin-pack all segments into
8*C lanes of length N (~195), which shortens the sequential recurrence
from 512 chain steps to ~195 while widening each step's tiles.  The
per-core kernel runs the recurrence in a transposed layout
[features, lanes] with bf16 matmuls/gates, the input projection and the
actor/critic heads overlapped with the sequential chain.  The host
permutes x/done in, un-permutes the outputs; this is exact (resets make
segments independent), not an approximation.

Self-contained: hardcodes all shapes; only depends on the platform's
concourse (Bass) library.
"""

import math
import os
import sys

import numpy as np

for _p in ("/opt/trn_rl_repo", os.path.expanduser("~/.axon_site/_ro/trn_rl_repo")):
    if os.path.isdir(_p) and _p not in sys.path:
        sys.path.insert(0, _p)
        break

import ml_dtypes

import concourse.bass as bass
import concourse.mybir as mybir
import concourse.tile as tile
from concourse import bacc

T, B, OBS, H, A, L = 512, 512, 64, 64, 6, 64
N_CORES = 8
C = 170                     # lanes (columns) per core
GS = 3                      # chain steps per group (GS*C <= 512 psum fp32)
COLS = GS * C               # 510
H3 = 3 * H
NLANES = N_CORES * C        # 1360
AO = A + 1                  # 7 outputs (6 logits + 1 value)

F32 = mybir.dt.float32
BF16 = mybir.dt.bfloat16
AF = mybir.ActivationFunctionType
ALU = mybir.AluOpType
BF = ml_dtypes.bfloat16

WEIGHT_KEYS = [
    "w_ih", "w_hh", "b_ih", "b_hh",
    "aw1", "ab1", "aw2", "ab2", "aw3", "ab3",
    "cw1", "cb1", "cw2", "cb2", "cw3", "cb3",
]


# --------------------------------------------------------------------------
# device kernel
# --------------------------------------------------------------------------

def build(nc, ng):
    """Emit the per-core kernel for ng groups (N = ng*GS chain steps)."""
    from contextlib import ExitStack

    n_steps = ng * GS

    xT_d = nc.dram_tensor("xT", [ng, OBS, COLS], BF16, kind="ExternalInput")
    mb_d = nc.dram_tensor("mb", [ng, H, COLS], BF16, kind="ExternalInput")
    h0T_d = nc.dram_tensor("h0T", [H, C], BF16, kind="ExternalInput")
    wih_d = nc.dram_tensor("w_ihT", [OBS, H3], BF16, kind="ExternalInput")
    whh_d = nc.dram_tensor("w_hhT", [H, H3], BF16, kind="ExternalInput")
    l1h_d = nc.dram_tensor("l1h", [64, 128], BF16, kind="ExternalInput")
    l1x_d = nc.dram_tensor("l1x", [64, 128], BF16, kind="ExternalInput")
    l2_d = nc.dram_tensor("l2", [128, 128], BF16, kind="ExternalInput")
    l3_d = nc.dram_tensor("l3", [128, AO], BF16, kind="ExternalInput")
    brz_d = nc.dram_tensor("b_rz", [128, 1], F32, kind="ExternalInput")
    bhhn_d = nc.dram_tensor("b_hhn", [H, 1], F32, kind="ExternalInput")
    bihn_d = nc.dram_tensor("b_ihn", [H, 1], F32, kind="ExternalInput")
    b1_d = nc.dram_tensor("b1", [128, 1], F32, kind="ExternalInput")
    b2_d = nc.dram_tensor("b2", [128, 1], F32, kind="ExternalInput")
    b3_d = nc.dram_tensor("b3", [AO, 1], F32, kind="ExternalInput")
    b3r_d = nc.dram_tensor("b3r", [1, AO], BF16, kind="ExternalInput")
    out_d = nc.dram_tensor("out", [ng, AO, COLS], F32, kind="ExternalOutput")

    with tile.TileContext(nc) as tc, ExitStack() as ctx:
        wp = ctx.enter_context(tc.tile_pool(name="wp", bufs=1))
        catp = ctx.enter_context(tc.tile_pool(name="catp", bufs=3))
        mbp = ctx.enter_context(tc.tile_pool(name="mbp", bufs=3))

        gatep = ctx.enter_context(tc.tile_pool(name="gatep", bufs=2))
        smallp = ctx.enter_context(tc.tile_pool(name="smallp", bufs=2))
        tmlp = ctx.enter_context(tc.tile_pool(name="tmlp", bufs=3))
        o7p = ctx.enter_context(tc.tile_pool(name="o7p", bufs=2))

        przp = ctx.enter_context(tc.tile_pool(name="przp", bufs=2, space="PSUM"))
        pginp = ctx.enter_context(tc.tile_pool(name="pginp", bufs=2, space="PSUM"))
        pghnp = ctx.enter_context(tc.tile_pool(name="pghnp", bufs=1, space="PSUM"))
        phd = ctx.enter_context(tc.tile_pool(name="phd", bufs=3, space="PSUM"))

        # ---- weights / biases (host-prepared, straight DMA loads) ----
        def load(dram, shape, dt, tag):
            t = wp.tile(shape, dt, tag=tag)
            nc.sync.dma_start(t[:], dram[:])
            return t

        wihT = load(wih_d, [OBS, H3], BF16, "wihT")
        whhT = load(whh_d, [H, H3], BF16, "whhT")
        h0T = load(h0T_d, [H, C], BF16, "h0T")
        l1h = load(l1h_d, [64, 128], BF16, "l1h")
        l1x = load(l1x_d, [64, 128], BF16, "l1x")
        l2 = load(l2_d, [128, 128], BF16, "l2")
        l3 = load(l3_d, [128, AO], BF16, "l3")
        b_r = wp.tile([H, 1], F32, tag="b_r")
        nc.sync.dma_start(b_r[:], brz_d[0:64])
        b_z = wp.tile([H, 1], F32, tag="b_z")
        nc.sync.dma_start(b_z[:], brz_d[64:128])
        b_hhn = load(bhhn_d, [H, 1], F32, "b_hhn")
        b_ihn = load(bihn_d, [H, 1], F32, "b_ihn")
        b1 = load(b1_d, [128, 1], F32, "b1")
        b2 = load(b2_d, [128, 1], F32, "b2")
        b3r = load(b3r_d, [1, AO], BF16, "b3r")
        ones_row = wp.tile([1, COLS], BF16, tag="ones_row")
        nc.vector.memset(ones_row[:], 1.0)

        def bulk(g):
            """x + mask loads, input-projection preloads for group g."""
            xT = catp.tile([OBS, COLS], BF16, tag="xT")
            nc.sync.dma_start(xT[:], xT_d[g])
            hsg = catp.tile([H, COLS], BF16, tag="hsg")
            mbt = mbp.tile([H, COLS], BF16, tag="mb")
            nc.sync.dma_start(mbt[:], mb_d[g])
            prz = przp.tile([128, COLS], F32, tag="prz")
            nc.tensor.matmul(
                prz[:], wihT[:, 0:128], xT[:],
                start=True, stop=False, skip_group_check=True,
            )
            pgin = pginp.tile([H, COLS], F32, tag="pgin")
            nc.tensor.matmul(
                pgin[:], wihT[:, 128:H3], xT[:], start=True, stop=True
            )
            return dict(xT=xT, hsg=hsg, mb=mbt, prz=prz, pgin=pgin)

        state = {}

        def chain(g, refs, refs_next, pieces=None):
            hsg, mbt, prz, pgin = refs["hsg"], refs["mb"], refs["prz"], refs["pgin"]
            for s in range(GS):
                t = g * GS + s
                cs = bass.ts(s, C)
                mh = state["mh"]
                nc.tensor.matmul(
                    prz[:, cs], whhT[:, 0:128], mh[:],
                    start=False, stop=(s == GS - 1), skip_group_check=True,
                )
                pghn = pghnp.tile([H, C], F32, tag="pghn")
                nc.tensor.matmul(
                    pghn[:], whhT[:, 128:H3], mh[:], start=True, stop=True
                )
                r = gatep.tile([H, C], BF16, tag="r")
                nc.scalar.activation(r[:], prz[0:64, cs], AF.Sigmoid, bias=b_r[:])
                z = gatep.tile([H, C], BF16, tag="z")
                nc.scalar.activation(z[:], prz[64:128, cs], AF.Sigmoid, bias=b_z[:])
                zm1 = smallp.tile([H, C], BF16, tag="zm1")
                nc.gpsimd.tensor_mul(zm1[:], z[:], mh[:])
                p = smallp.tile([H, C], BF16, tag="p")
                nc.vector.scalar_tensor_tensor(
                    p[:], pghn[:], b_hhn[:], r[:], ALU.add, ALU.mult
                )
                q = smallp.tile([H, C], BF16, tag="q")
                nc.vector.tensor_add(q[:], p[:], pgin[:, cs])
                n = smallp.tile([H, C], BF16, tag="n")
                nc.scalar.activation(n[:], q[:], AF.Tanh, bias=b_ihn[:])
                vb = smallp.tile([H, C], BF16, tag="vb")
                nc.vector.scalar_tensor_tensor(
                    vb[:], z[:], 1.0, n[:], ALU.subtract, ALU.mult
                )
                nc.vector.tensor_sub(hsg[:, cs], zm1[:], vb[:])
                if t < n_steps - 1:
                    mh2 = smallp.tile([H, C], BF16, tag="mh")
                    if s == GS - 1:
                        mbn = refs_next["mb"][:, 0:C]
                    else:
                        mbn = mbt[:, bass.ts(s + 1, C)]
                    nc.vector.tensor_mul(mh2[:], hsg[:, cs], mbn)
                    state["mh"] = mh2
                if pieces is not None:
                    pieces[s]()             # head work for group g-1, post-step

        def head_pieces(g, refs):
            """Head MLP for group g split into GS pieces for interleaving."""
            hsg, xT = refs["hsg"], refs["xT"]
            st = {}

            def piece0():
                p1 = phd.tile([128, COLS], F32, tag="ph")
                nc.tensor.matmul(p1[:], l1h[:], hsg[:], start=True, stop=False,
                                 skip_group_check=True)
                nc.tensor.matmul(p1[:], l1x[:], xT[:], start=False, stop=True,
                                 skip_group_check=True)
                st["p1"] = p1

            def piece1():
                t1 = tmlp.tile([128, COLS], BF16, tag="t1")
                nc.scalar.activation(t1[:], st["p1"][:], AF.Tanh, bias=b1[:])
                p2 = phd.tile([128, COLS], F32, tag="ph")
                nc.tensor.matmul(p2[:], l2[:], t1[:], start=True, stop=True)
                st["p2"] = p2

            def piece2():
                t2 = tmlp.tile([128, COLS], BF16, tag="t2")
                nc.scalar.activation(t2[:], st["p2"][:], AF.Tanh, bias=b2[:])
                p3 = phd.tile([128, COLS], F32, tag="ph")
                nc.tensor.matmul(p3[:AO, :], b3r[:], ones_row[:],
                                 start=True, stop=False, skip_group_check=True)
                nc.tensor.matmul(p3[:AO, :], l3[:], t2[:],
                                 start=False, stop=True, skip_group_check=True)
                o7 = o7p.tile([AO, COLS], F32, tag="o7")
                half = COLS // 2
                nc.vector.tensor_copy(o7[:, 0:half], p3[:AO, 0:half])
                nc.vector.tensor_copy(o7[:, half:COLS], p3[:AO, half:COLS])
                nc.sync.dma_start(out_d[g], o7[:])

            return [piece0, piece1, piece2]

        refs = bulk(0)
        mh0 = smallp.tile([H, C], BF16, tag="mh")
        nc.vector.tensor_mul(mh0[:], h0T[:], refs["mb"][:, 0:C])
        state["mh"] = mh0
        pieces = None
        for g in range(1, ng):
            refs_next = bulk(g)
            chain(g - 1, refs, refs_next, pieces)
            pieces = head_pieces(g - 1, refs)
            refs = refs_next
        chain(ng - 1, refs, None, pieces)
        for pc in head_pieces(ng - 1, refs):
            pc()

    return nc


_BUILT = {}


def get_built(ng):
    if ng not in _BUILT:
        nc = bacc.Bacc(None, target_bir_lowering=False)
        build(nc, ng)
        nc.compile()
        _BUILT[ng] = nc
    return _BUILT[ng]


# --------------------------------------------------------------------------
# host-side packing
# --------------------------------------------------------------------------

def _enumerate_segments(done2):
    """done2 [T,B] -> (seg_env, seg_t0, seg_len) with cuts at done==1.0."""
    starts = done2 == 1.0
    starts[0, :] = True
    nseg_per_env = starts.sum(axis=0)
    seg_env = np.repeat(np.arange(done2.shape[1]), nseg_per_env)
    env_idx, t_idx = np.nonzero(starts.T)
    seg_t0 = t_idx
    # length = next start - this start (within env)
    seg_len = np.empty(len(seg_t0), np.int64)
    pos = 0
    Tn = done2.shape[0]
    for b, k in enumerate(nseg_per_env):
        ts = seg_t0[pos:pos + k]
        seg_len[pos:pos + k - 1] = np.diff(ts)
        seg_len[pos + k - 1] = Tn - ts[-1]
        pos += k
    return seg_env, seg_t0, seg_len


def _pack(done2, h0_nonzero):
    """Bin-pack segments into NLANES lanes.  Returns (N, lane, off) per seg."""
    seg_env, seg_t0, seg_len = _enumerate_segments(done2)
    nseg = len(seg_len)
    total = int(seg_len.sum())
    cap = max(int(seg_len.max()), math.ceil(total / NLANES))
    cap = ((cap + GS - 1) // GS) * GS

    while True:
        rem = np.full(NLANES, cap, np.int64)
        lane = np.full(nseg, -1, np.int64)
        off = np.zeros(nseg, np.int64)
        ok = True
        if h0_nonzero:
            # t=0 segments that continue from h0 must sit at a lane start
            pin = np.nonzero((seg_t0 == 0) & (done2[0, seg_env] != 1.0))[0]
            if len(pin) > NLANES:
                raise RuntimeError("too many h0 segments")
            for j, i in enumerate(pin):
                if seg_len[i] > cap:
                    ok = False
                    break
                lane[i] = j
                off[i] = 0
                rem[j] = cap - seg_len[i]
        if ok:
            order = np.argsort(-seg_len, kind="stable")
            for i in order:
                if lane[i] >= 0:
                    continue
                Lg = seg_len[i]
                j = int(np.argmax(rem >= Lg))
                if rem[j] < Lg:
                    ok = False
                    break
                lane[i] = j
                off[i] = cap - rem[j]
                rem[j] -= Lg
        if ok:
            return cap, seg_env, seg_t0, seg_len, lane, off
        cap += GS


def _prepare(inputs):
    """Host-side pack + permute.  Returns (ng, in_maps, gather_idx)."""
    x = np.ascontiguousarray(np.asarray(inputs["x"], np.float32)).reshape(T, B, OBS)
    done2 = np.ascontiguousarray(
        np.asarray(inputs["done"], np.float32)
    ).reshape(T, B)
    h0 = np.ascontiguousarray(
        np.asarray(inputs["gru_state"], np.float32)
    ).reshape(B, H)
    h0_nonzero = bool(np.any(h0))

    N, seg_env, seg_t0, seg_len, seg_lane, seg_off = _pack(done2, h0_nonzero)
    ng = N // GS

    # flat (src slot) -> (dst slot) index arrays
    reps = seg_len
    src_env = np.repeat(seg_env, reps)
    within = np.concatenate([np.arange(l) for l in seg_len])
    src_t = np.repeat(seg_t0, reps) + within
    dst_lane = np.repeat(seg_lane, reps)
    dst_n = np.repeat(seg_off, reps) + within

    # packed done: copy source done values; padding slots = 1 (reset)
    donep = np.ones((N, NLANES), np.float32)
    donep[dst_n, dst_lane] = done2[src_t, src_env]
    if not h0_nonzero:
        # zero h0 == reset; break any dependence on prior lane garbage
        startmask = within == 0
        donep[dst_n[startmask], dst_lane[startmask]] = 1.0
    mbp_ = (1.0 - donep).astype(BF)                       # [N, NLANES]

    xp = np.zeros((N, NLANES, OBS), BF)
    xp[dst_n, dst_lane] = x[src_t, src_env].astype(BF)

    h0T_all = np.zeros((H, NLANES), BF)
    if h0_nonzero:
        first = within == 0
        fl = dst_lane[first]
        fe = src_env[first]
        fn = dst_n[first]
        sel = fn == 0
        h0T_all[:, fl[sel]] = h0[fe[sel]].T.astype(BF)

    # weights, host-transformed
    w_ih = np.asarray(inputs["w_ih"], np.float32)
    w_hh = np.asarray(inputs["w_hh"], np.float32)
    b_ih = np.asarray(inputs["b_ih"], np.float32)
    b_hh = np.asarray(inputs["b_hh"], np.float32)
    l1 = np.concatenate(
        [np.asarray(inputs["aw1"], np.float32).T,
         np.asarray(inputs["cw1"], np.float32).T], axis=1)       # [128,128]
    l2 = np.zeros((128, 128), np.float32)
    l2[0:64, 0:64] = np.asarray(inputs["aw2"], np.float32).T
    l2[64:128, 64:128] = np.asarray(inputs["cw2"], np.float32).T
    l3 = np.zeros((128, AO), np.float32)
    l3[0:64, 0:A] = np.asarray(inputs["aw3"], np.float32).T
    l3[64:128, A:AO] = np.asarray(inputs["cw3"], np.float32).T
    brz = (b_ih[0:128] + b_hh[0:128]).reshape(128, 1)
    bhhn = b_hh[128:H3].reshape(H, 1)
    bihn = b_ih[128:H3].reshape(H, 1)
    b1 = np.concatenate(
        [np.asarray(inputs["ab1"], np.float32),
         np.asarray(inputs["cb1"], np.float32)]).reshape(128, 1)
    b2 = np.concatenate(
        [np.asarray(inputs["ab2"], np.float32),
         np.asarray(inputs["cb2"], np.float32)]).reshape(128, 1)
    b3 = np.concatenate(
        [np.asarray(inputs["ab3"], np.float32),
         np.asarray(inputs["cb3"], np.float32)]).reshape(AO, 1)

    common = {
        "b3r": np.ascontiguousarray(b3.reshape(1, AO).astype(BF)),
        "w_ihT": np.ascontiguousarray(w_ih.T.astype(BF)),
        "w_hhT": np.ascontiguousarray(w_hh.T.astype(BF)),
        "l1h": np.ascontiguousarray(l1[0:64].astype(BF)),
        "l1x": np.ascontiguousarray(l1[64:128].astype(BF)),
        "l2": l2.astype(BF), "l3": l3.astype(BF),
        "b_rz": brz, "b_hhn": bhhn, "b_ihn": bihn,
        "b1": b1, "b2": b2, "b3": b3,
    }

    in_maps = []
    for c in range(N_CORES):
        sl = slice(c * C, (c + 1) * C)
        # [N, C, OBS] -> [ng, OBS, GS*C] with column order (s, lane)
        xc = xp[:, sl, :].reshape(ng, GS, C, OBS).transpose(0, 3, 1, 2)
        mc = mbp_[:, sl].reshape(ng, GS, C)
        mcb = np.broadcast_to(mc[:, None, :, :], (ng, H, GS, C))
        m = dict(common)
        m["xT"] = np.ascontiguousarray(xc.reshape(ng, OBS, COLS))
        m["mb"] = np.ascontiguousarray(mcb.reshape(ng, H, COLS))
        m["h0T"] = np.ascontiguousarray(h0T_all[:, sl])
        in_maps.append(m)

    gather = (src_t, src_env, dst_n, dst_lane, N, ng)
    return ng, in_maps, gather


def _assemble(per_core_outs, gather):
    src_t, src_env, dst_n, dst_lane, N, ng = gather
    # per-core out [ng, AO, COLS] -> [N, C, AO]
    packed = np.concatenate(
        [
            np.asarray(o, np.float32)
            .reshape(ng, AO, GS, C)
            .transpose(0, 2, 3, 1)
            .reshape(N, C, AO)
            for o in per_core_outs
        ],
        axis=1,
    )                                                     # [N, NLANES, AO]
    full = np.empty((T * B, AO), np.float32)
    full[src_t * B + src_env] = packed[dst_n, dst_lane]
    return full


def run_on_hw(inputs, trace=False, **kw):
    from concourse.bass_utils import run_bass_kernel_spmd

    ng, in_maps, gather = _prepare(inputs)
    nc = get_built(ng)
    res = run_bass_kernel_spmd(
        nc, in_maps, core_ids=list(range(N_CORES)), trace=trace, **kw
    )
    out = _assemble([r["out"] for r in res.results], gather)
    return out, res


def kernel(**inputs):
    out, _ = run_on_hw(inputs)
    return out
